# revision 1
# baseline (speedup 1.0000x reference)
"""GAT 2-layer kernel for 8 trn2 NeuronCores (self-contained).

Sharding: destination-node blocks across 8 cores. Per core: 49 blocks x 128
nodes. Layer GEMMs computed on owned nodes; per-node attention factors
(u=exp(a_src), v=exp(0.2 a_src), and dst-side) folded into gatherable row
tables (Hcat/Gcat) that are AllGathered; edge phases gather rows by src via
dma_gather and do segment-softmax-aggregation with per-chunk selection-matrix
matmuls (exp(leaky(x)) == max(exp(x), exp(0.2 x)) makes the logits separable).
Edge slots are split by source half (int16 gather indices); both layers share
the same slot assignment so one selection-matrix pair and one index table
serve both edge phases.
"""
import sys, types
sys.path.insert(0, '/opt/trn_rl_repo')

import numpy as np
import ml_dtypes

# ---------------- problem constants (hardcoded) ----------------
N = 50000
F_IN = 512
HID = 64
HEADS = 8
CLS = 64
NEG_SLOPE = 0.2
NCORES = 8
NPC = 6250
NB = 49
BLK = 128
NLOC = NB * BLK           # 6272
NGLOB = NCORES * NLOC     # 50176
CPS = 6                   # chunks per side per block
SIDE = CPS * 128          # 768 edge slots per side
CH1 = 2 * CPS             # 12 chunks per block
WCI = SIDE // 16          # 48 idx cols per side
HROW = 640                # Hcat row (bf16)
GROW = 128                # Gcat row (bf16)
AGS0 = 25                 # blocks per core in side 0
R0 = AGS0 * BLK           # 3200 rows per core in side 0
R1 = NLOC - R0            # 3072 rows per core in side 1
S0TOT = NCORES * R0       # 25600 rows in side 0
S1TOT = NCORES * R1       # 24576 rows in side 1
HCH = [(0, 13), (13, 25), (25, 37), (37, 49)]  # AG chunks (block ranges)
HBASE = []                # global row base of each AG chunk
_acc = 0
for _lo, _hi in HCH:
    HBASE.append(_acc)
    _acc += NCORES * (_hi - _lo) * BLK
EPS = 1e-16


def _install_ntff_hook():
    if 'antenv.axon_hooks' in sys.modules:
        return
    try:
        sys.path.insert(0, '/root/.axon_site')
        from trn_agent_boot.trn_boot import _ntff_profile_via_ctypes
        hook = _ntff_profile_via_ctypes('/opt/axon/libaxon_pjrt.so')
    except Exception:
        return
    mod = types.ModuleType('antenv.axon_hooks')
    mod._hook = hook
    mod.get_axon_ntff_profile_hook = lambda: mod._hook
    mod.set_axon_ntff_profile_hook = lambda h: setattr(mod, '_hook', h)
    sys.modules['antenv.axon_hooks'] = mod


_install_ntff_hook()

import concourse.bass as bass
import concourse.mybir as mybir
import concourse.tile as tile
from concourse import library_config
from concourse.bass_utils import run_bass_kernel_spmd
from concourse.vector_clock import VectorClock, ScopedClock

bf16 = mybir.dt.bfloat16
f32 = mybir.dt.float32
fp8 = mybir.dt.float8e4

# ------------- tile framework patches (walrus: 1 sync wait / inst) ---------


def _drain_and_barrier_split(self, tick_clock, wait_clock):
    nc = self.nc
    full = tick_clock.global_clock
    procs = [p for p in range(27) if full[p] > 0]
    for p in procs:
        sub = VectorClock([full[q] if q == p else 0 for q in range(27)])
        drain_inst = nc.sync.drain(fusable=False)
        wait_clock.add_sem_waits(drain_inst.ins, ScopedClock({None: sub}))
    if not procs:
        nc.sync.drain(fusable=False)
    nc.all_engine_barrier()
    assert self.sems is not None
    popped = nc._tile_sem_poison_stack.pop()
    assert popped is self._sem_poison
    nc.clear_and_free_semaphores(list(self.sems.allocated().values()))
    nc.all_engine_barrier()


def _split_excess_waits(nc):
    for bb in nc.main_func.blocks:
        insts = bb.instructions
        i = 0
        while i < len(insts):
            ins = insts[i]
            si = ins.sync_info
            if si is None:
                i += 1
                continue
            waits = list(si.on_wait)
            if len(waits) <= 1:
                i += 1
                continue
            keep, surplus = waits[:1], waits[1:]
            ins.sync_info = mybir.SyncInfo(on_wait=keep, on_update=list(si.on_update))
            nops = []
            for w in surplus:
                nop = mybir.InstNoOp(name=nc.get_next_instruction_name())
                nop.engine = ins.engine
                nop.sync_info = mybir.SyncInfo(on_wait=[w], on_update=[])
                nc.register_instruction(nop)
                nops.append(nop)
            for k, nop in enumerate(nops):
                insts.insert(i + k, nop)
            i += 1 + len(nops)


_PATCHED = False


def _install_patches():
    global _PATCHED
    if _PATCHED:
        return
    _orig_exit = tile.TileContext.__exit__

    def _exit_with_split(self, *a):
        r = _orig_exit(self, *a)
        _split_excess_waits(self.nc)
        return r

    tile.TileContext._drain_and_barrier = _drain_and_barrier_split
    tile.TileContext.__exit__ = _exit_with_split
    _PATCHED = True


# ---------------- host-side graph preprocessing ----------------


def _wrap16(flat_idx):
    W = len(flat_idx) // 16
    t = np.asarray(flat_idx, np.int16).reshape(W, 16).T
    return np.tile(t, (8, 1))


def _host_prep(x, edge_index, W1, att_src1, att_dst1, b1, W2, att_src2,
               att_dst2, b2):
    src_o = np.concatenate([np.asarray(edge_index[0]),
                            np.arange(N, dtype=np.int64)]).astype(np.int64)
    dst_o = np.concatenate([np.asarray(edge_index[1]),
                            np.arange(N, dtype=np.int64)]).astype(np.int64)

    core_of = dst_o // NPC
    deg = np.bincount(dst_o, minlength=N)

    gid = np.full(N, -1, np.int64)
    node_of_gid = np.full(NGLOB, -1, np.int64)
    for c in range(NCORES):
        nodes = np.arange(c * NPC, (c + 1) * NPC)
        d = deg[nodes]
        order = np.argsort(-d, kind='stable')
        blk_cnt = np.zeros(NB, np.int64)
        blk_load = np.zeros(NB, np.int64)
        assign = np.full(NPC, -1, np.int64)
        for i in order:
            b = int(np.argmin(blk_load + (blk_cnt >= BLK) * (1 << 40)))
            assign[i] = b
            blk_cnt[b] += 1
            blk_load[b] += d[i]
        slot_ctr = np.zeros(NB, np.int64)
        for i in range(NPC):
            b = assign[i]
            g = c * NLOC + b * BLK + slot_ctr[b]
            slot_ctr[b] += 1
            gid[nodes[i]] = g
            node_of_gid[g] = nodes[i]

    src_g = gid[src_o]
    dst_g = gid[dst_o]
    dst_block = (dst_g % NLOC) // BLK
    dst_slot = dst_g % BLK
    src_core = src_g // NLOC
    src_row = src_g % NLOC
    src_blk = src_row // BLK
    side = (src_row >= R0).astype(np.int64)
    src_gidx = np.zeros_like(src_g)
    for _r, (_lo, _hi) in enumerate(HCH):
        _m = (src_blk >= _lo) & (src_blk < _hi)
        _sz = (_hi - _lo) * BLK
        src_gidx[_m] = (HBASE[_r] + src_core[_m] * _sz
                        + (src_row[_m] - _lo * BLK))
    src_sidx = np.where(side == 0, src_gidx, src_gidx - S0TOT)

    idxL1 = np.full((NCORES, NB, 2, SIDE), -1, np.int32)
    dslL1 = np.full((NCORES, NB, 2, SIDE), -1, np.int32)
    cntL1 = np.zeros((NCORES, NB, 2), np.int64)

    for c in range(NCORES):
        em = np.nonzero(core_of == c)[0]
        eb = dst_block[em]
        for b in range(NB):
            eidx = em[eb == b]
            dslots = dst_slot[eidx]
            sides = side[eidx]
            for s in (0, 1):
                ms = sides == s
                k = int(ms.sum())
                if k > SIDE:
                    raise RuntimeError(f"L1 overflow c{c} b{b} s{s}: {k}")
                idxL1[c, b, s, :k] = src_sidx[eidx][ms]
                dslL1[c, b, s, :k] = dslots[ms]
                cntL1[c, b, s] = k

    # SPMD: same valid-descriptor count on every core -> pad with idx 0
    nvmax = cntL1.max(axis=0)      # [NB, 2]
    nvL1 = np.minimum(((nvmax + 63) // 64) * 64, SIDE)
    for c in range(NCORES):
        for b in range(NB):
            for s in (0, 1):
                k = int(cntL1[c, b, s])
                idxL1[c, b, s, k:int(nvL1[b, s])] = 0

    SL1 = np.zeros((NCORES, NB, 128, CH1 * 128), np.float32)
    STL1 = np.zeros((NCORES, NB, 128, CH1 * 128), np.float32)
    for c in range(NCORES):
        for b in range(NB):
            for s in (0, 1):
                for j in range(CPS):
                    ch = s * CPS + j
                    sl = dslL1[c, b, s, j * 128:(j + 1) * 128]
                    e_i = np.nonzero(sl >= 0)[0]
                    n_i = sl[e_i]
                    SL1[c, b, e_i, ch * 128 + n_i] = 1
                    STL1[c, b, n_i, ch * 128 + e_i] = 1
    SB1 = np.concatenate([SL1, STL1], axis=-1).astype(
        ml_dtypes.float8_e4m3).view(np.int8)  # [NCORES, NB, 128, 2*CH1*128]

    W1 = np.asarray(W1, np.float32)
    att_src1 = np.asarray(att_src1, np.float32)
    att_dst1 = np.asarray(att_dst1, np.float32)
    b1 = np.asarray(b1, np.float32)
    W2 = np.asarray(W2, np.float32)
    att_src2 = np.asarray(att_src2, np.float32)
    att_dst2 = np.asarray(att_dst2, np.float32)
    b2 = np.asarray(b2, np.float32)

    Msrc = np.zeros((F_IN, HEADS), np.float32)
    Mdst = np.zeros((F_IN, HEADS), np.float32)
    for h in range(HEADS):
        Msrc[h * HID:(h + 1) * HID, h] = att_src1[h]
        Mdst[h * HID:(h + 1) * HID, h] = att_dst1[h]
    # h features stored head-major (natural W1 layout: col = head*64 + ch)
    W1aug = np.zeros((F_IN + 128, F_IN + 16), np.float32)
    W1aug[:F_IN, 0:F_IN] = W1
    W1aug[:F_IN, F_IN:F_IN + 8] = W1 @ Msrc
    W1aug[:F_IN, F_IN + 8:F_IN + 16] = W1 @ Mdst
    W1aug[F_IN, 0:F_IN] = b1
    W2aug = np.zeros((F_IN, 69), np.float32)
    W2aug[:, 0:CLS] = W2
    W2aug[:, 64] = W2 @ att_src2[0]
    W2aug[:, 65] = W2 @ att_dst2[0]
    W2bias = np.zeros((128, 69), np.float32)
    W2bias[0, 0:CLS] = b2
    W2bias[0, 68] = 1.0

    x = np.asarray(x, np.float32)
    xTs = []
    for c in range(NCORES):
        cols = node_of_gid[c * NLOC:(c + 1) * NLOC]
        validc = cols >= 0
        xc = np.zeros((NLOC, F_IN), np.float32)
        xc[validc] = x[cols[validc]]
        xt = np.zeros((F_IN + 128, NLOC), np.float32)
        xt[:F_IN] = xc.T
        xt[F_IN] = 1.0
        xTs.append(xt.astype(ml_dtypes.bfloat16))

    idxL1_dev = np.zeros((NCORES, 128, NB * 2 * WCI), np.int16)
    for c in range(NCORES):
        for b in range(NB):
            for s in (0, 1):
                col = (b * 2 + s) * WCI
                idxL1_dev[c, :, col:col + WCI] = _wrap16(idxL1[c, b, s, :])

    in_maps = []
    W1aug_bf = W1aug.astype(ml_dtypes.bfloat16)
    W2aug_bf = W2aug.astype(ml_dtypes.bfloat16)
    W2bias_bf = W2bias.astype(ml_dtypes.bfloat16)
    for c in range(NCORES):
        in_maps.append({
            "xT": np.ascontiguousarray(xTs[c]),
            "W1aug": W1aug_bf,
            "W2aug": W2aug_bf,
            "W2bias": W2bias_bf,
            "idxL1": np.ascontiguousarray(idxL1_dev[c]),
            "SB1": np.ascontiguousarray(SB1[c]),
        })
    meta = {"node_of_gid": node_of_gid, "nvL1": nvL1}
    return in_maps, meta


# ---------------- device program ----------------


def _build_program(nvL1):
    _install_patches()
    nc = bass.Bass(num_swdge_queues=4)
    AF = mybir.ActivationFunctionType
    OP = mybir.AluOpType
    KW = (F_IN + 128) // 128          # 5 k-chunks for GEMM1
    WROW = F_IN + 16                  # 528 W1aug cols

    xT = nc.dram_tensor("xT", [F_IN + 128, NLOC], bf16, kind="ExternalInput")
    W1a = nc.dram_tensor("W1aug", [F_IN + 128, WROW], bf16, kind="ExternalInput")
    W2a = nc.dram_tensor("W2aug", [F_IN, 69], bf16, kind="ExternalInput")
    W2b = nc.dram_tensor("W2bias", [128, 69], bf16, kind="ExternalInput")
    idxL1 = nc.dram_tensor("idxL1", [128, NB * 2 * WCI], mybir.dt.int16,
                           kind="ExternalInput")
    SB1 = nc.dram_tensor("SB1", [NB, 128, 2 * CH1 * 128],
                         mybir.dt.int8, kind="ExternalInput")

    out_cat = nc.dram_tensor("out_cat", [NLOC, 128], f32, kind="ExternalOutput")

    Hcat_loc = nc.dram_tensor("Hcat_loc", [NLOC, HROW], bf16)
    Hcat_g = nc.dram_tensor("Hcat_g", [NGLOB, HROW], bf16, addr_space="Shared")
    Gcat_loc = nc.dram_tensor("Gcat_loc", [NLOC, GROW], bf16)
    Gcat_g = nc.dram_tensor("Gcat_g", [NGLOB, GROW], bf16, addr_space="Shared")

    groups = [list(range(NCORES))]

    with tile.TileContext(nc) as tc:
        with tc.tile_critical():
            nc.gpsimd.load_library(library_config.mlp)
        nvset = sorted({int(v) for v in nvL1.flatten()})
        nvregs = {}
        for v in nvset:
            if v > 0:
                nvregs[v] = nc.gpsimd.to_reg(v)

        with tc.tile_pool(name="const", bufs=1) as constp:
            w1t = constp.tile([128, KW * WROW], bf16)
            for k in range(KW):
                nc.sync.dma_start(w1t[:, k * WROW:(k + 1) * WROW],
                                  W1a[k * 128:(k + 1) * 128, :])
            w2t = constp.tile([128, 4 * 69], bf16)
            for k in range(4):
                nc.sync.dma_start(w2t[:, k * 69:(k + 1) * 69],
                                  W2a[k * 128:(k + 1) * 128, :])
            w2bt = constp.tile([128, 69], bf16)
            nc.sync.dma_start(w2bt[:], W2b[:, :])
            e0ones = constp.tile([128, 128], bf16)
            nc.vector.memset(e0ones[:], 0.0)
            nc.vector.memset(e0ones[0:1, :], 1.0)
            ident = constp.tile([128, 128], bf16)
            from concourse.masks import make_identity
            make_identity(nc, ident[:])
            dt_all = constp.tile([128, NB * 16], bf16)
            dt2_all = constp.tile([128, NB * 2], bf16)
            idx1t = constp.tile([128, NB * 2 * WCI], mybir.dt.int16)
            nc.sync.dma_start(idx1t[:], idxL1[:, :])

            # ---- phase 1: GEMM1 + Hcat rows ----
            with tc.tile_pool(name="p1", bufs=3) as p1, \
                 tc.tile_pool(name="ps1a", bufs=2, space="PSUM") as ps1a, \
                 tc.tile_pool(name="ps1b", bufs=2, space="PSUM") as ps1b:
                for b in range(NB):
                    pA = ps1a.tile([128, F_IN], f32)
                    pB = ps1b.tile([128, 16], f32)
                    xt = p1.tile([128, KW * 128], bf16, tag="xt")
                    nc.sync.dma_start(
                        xt[:].rearrange("p (k j) -> p k j", k=KW),
                        xT[0:KW * 128, b * 128:(b + 1) * 128].rearrange(
                            "(k p) j -> p k j", k=KW))
                    for k in range(KW):
                        nc.tensor.matmul(pA[:], xt[:, k * 128:(k + 1) * 128],
                                         w1t[:, k * WROW:k * WROW + F_IN],
                                         start=(k == 0), stop=(k == KW - 1))
                        nc.tensor.matmul(pB[:], xt[:, k * 128:(k + 1) * 128],
                                         w1t[:, k * WROW + F_IN:(k + 1) * WROW],
                                         start=(k == 0), stop=(k == KW - 1))
                    hc = p1.tile([128, HROW], bf16, tag="hc")
                    nc.scalar.activation(hc[:, 0:F_IN], pA[:], AF.Copy)
                    nc.scalar.activation(hc[:, 512:520], pB[:, 0:8], AF.Exp)
                    nc.scalar.activation(hc[:, 520:528], pB[:, 0:8], AF.Exp,
                                         scale=NEG_SLOPE)
                    nc.scalar.activation(hc[:, 528:536], pB[:, 8:16], AF.Exp)
                    nc.scalar.activation(hc[:, 536:544], pB[:, 8:16], AF.Exp,
                                         scale=NEG_SLOPE)
                    if b < 3:
                        nc.vector.memset(hc[:, 544:HROW], 0.0)
                    nc.vector.tensor_copy(dt_all[:, b * 16:(b + 1) * 16],
                                          hc[:, 528:544])
                    nc.scalar.dma_start(Hcat_loc[b * 128:(b + 1) * 128, :], hc[:])
                    for r, (lo, hi) in enumerate(HCH[:-1]):
                        if b == hi - 1:
                            nc.gpsimd.collective_compute(
                                "AllGather", mybir.AluOpType.bypass,
                                replica_groups=groups,
                                ins=[Hcat_loc[lo * BLK:hi * BLK, :]],
                                outs=[Hcat_g[HBASE[r]:HBASE[r] + NCORES
                                             * (hi - lo) * BLK, :]])

            # ---- phase 2: AllGather Hcat (tail chunk) ----
            lo, hi = HCH[-1]
            nc.gpsimd.collective_compute(
                "AllGather", mybir.AluOpType.bypass, replica_groups=groups,
                ins=[Hcat_loc[lo * BLK:hi * BLK, :]],
                outs=[Hcat_g[HBASE[-1]:NGLOB, :]])

            # ---- phase 3: L1 edges + block tails + GEMM2 + Gcat ----
            # 2-stage software pipeline: stage1(b) = gather + edge matmuls,
            # stage2(b) = normalize + elu + GEMM2 + Gcat row; stage2(b) is
            # emitted after stage1(b+1) so in-order engines overlap blocks.
            FEAT = CH1 * F_IN           # hs feature region size
            with tc.tile_pool(name="p3", bufs=3) as p3, \
                 tc.tile_pool(name="pg3", bufs=1) as pg3, \
                 tc.tile_pool(name="p3s", bufs=2) as p3s, \
                 tc.tile_pool(name="psA", bufs=2, space="PSUM") as psA, \
                 tc.tile_pool(name="psB", bufs=2, space="PSUM") as psB, \
                 tc.tile_pool(name="psuv", bufs=1, space="PSUM") as psuv, \
                 tc.tile_pool(name="pst", bufs=1, space="PSUM") as pst, \
                 tc.tile_pool(name="ps2", bufs=1, space="PSUM") as ps2:
                pAB = {}

                def stage1(b):
                    sbst = p3s.tile([128, 2 * CH1 * 128], mybir.dt.int8,
                                    tag="sbst")
                    nc.sync.dma_start(sbst[:], SB1[b, :, :])
                    sb = sbst[:, 0:CH1 * 128].bitcast(fp8)
                    stb = sbst[:, CH1 * 128:2 * CH1 * 128].bitcast(fp8)
                    gl = []
                    for s in (0, 1):
                        g = pg3.tile([128, CPS * HROW], bf16, tag=f"g{s}{b % 3}")
                        gv = g[:].rearrange("p (c w) -> p c w", w=HROW)
                        if b < 3:
                            nc.vector.memset(g[:], 0.0)
                        nvs = int(nvL1[b, s])
                        if nvs > 0:
                            nc.gpsimd.dma_gather(
                                out_ap=gv[:, :, :],
                                in_ap=(Hcat_g[0:S0TOT, :] if s == 0 else
                                       Hcat_g[S0TOT:NGLOB, :]),
                                idxs_ap=idx1t[:, (b * 2 + s) * WCI:
                                              (b * 2 + s + 1) * WCI],
                                num_idxs=SIDE, num_idxs_reg=nvregs[nvs],
                                elem_size=HROW, queue_num=(b * 2 + s) % 4)
                        gl.append(g)
                    puv = psuv.tile([128, CH1 * 16], f32)
                    for c in range(CH1):
                        nc.tensor.matmul(puv[:, c * 16:(c + 1) * 16],
                                         stb[:, c * 128:(c + 1) * 128],
                                         dt_all[:, b * 16:(b + 1) * 16],
                                         start=True, stop=True)
                    uvb = p3.tile([128, CH1 * 16], bf16, tag="uvb")
                    nc.scalar.activation(uvb[:], puv[:], AF.Copy)
                    pA = psA.tile([128, F_IN], f32)
                    pB = psB.tile([128, 8], f32)
                    # hs: features [0:FEAT] chunk-major, exb tail [FEAT:FEAT+96]
                    hs = p3.tile([128, FEAT + CH1 * 8], bf16, tag="hs")
                    uvs = uvb[:].rearrange("p (c w) -> p c w", w=16)
                    for s in (0, 1):
                        g3 = gl[s][:].rearrange("p (c w) -> p c w", w=HROW)
                        tU = p3.tile([128, CPS * 8], f32, tag="tU")
                        tV = p3.tile([128, CPS * 8], f32, tag="tV")
                        exb = hs[:, FEAT + s * CPS * 8:FEAT + (s + 1) * CPS * 8]
                        nc.vector.tensor_tensor(
                            out=tU[:].rearrange("p (c w) -> p c w", w=8),
                            in0=g3[:, :, 512:520],
                            in1=uvs[:, s * CPS:(s + 1) * CPS, 0:8],
                            op=OP.mult)
                        nc.vector.tensor_tensor(
                            out=tV[:].rearrange("p (c w) -> p c w", w=8),
                            in0=g3[:, :, 520:528],
                            in1=uvs[:, s * CPS:(s + 1) * CPS, 8:16],
                            op=OP.mult)
                        nc.vector.tensor_tensor(out=exb, in0=tU[:], in1=tV[:],
                                                op=OP.max)
                        nc.vector.tensor_tensor(
                            out=hs[:, s * CPS * F_IN:(s + 1) * CPS * F_IN]
                            .rearrange("p (c h w) -> p c h w", h=HEADS, w=HID),
                            in0=g3[:, :, 0:F_IN].rearrange(
                                "p c (h w) -> p c h w", h=HEADS),
                            in1=exb.rearrange("p (c h o) -> p c h o",
                                              h=HEADS, o=1).to_broadcast(
                                [128, CPS, HEADS, HID]),
                            op=OP.mult)
                    for c in range(CH1):
                        nc.tensor.matmul(pA[:], sb[:, c * 128:(c + 1) * 128],
                                         hs[:, c * F_IN:(c + 1) * F_IN],
                                         start=(c == 0), stop=(c == CH1 - 1))
                        nc.tensor.matmul(pB[:], sb[:, c * 128:(c + 1) * 128],
                                         hs[:, FEAT + c * 8:FEAT + (c + 1) * 8],
                                         start=(c == 0), stop=(c == CH1 - 1))
                    pAB[b] = (pA, pB)

                def stage2(b):
                    pA, pB = pAB.pop(b)
                    sden = p3.tile([128, 8], f32, tag="sden")
                    nc.vector.tensor_scalar_add(sden[:], pB[:], EPS)
                    rec = p3.tile([128, 8], f32, tag="rec")
                    nc.vector.reciprocal(rec[:], sden[:])
                    o1 = p3.tile([128, F_IN], f32, tag="o1")
                    nc.vector.tensor_tensor(
                        out=o1[:].rearrange("p (h w) -> p h w", h=HEADS),
                        in0=pA[:].rearrange("p (h w) -> p h w", h=HEADS),
                        in1=rec[:].rearrange("p (h o) -> p h o", o=1).to_broadcast(
                            [128, HEADS, HID]),
                        op=OP.mult)
                    rneg = p3.tile([128, F_IN], f32, tag="rneg")
                    nc.scalar.activation(rneg[:], o1[:], AF.Relu, scale=-1.0)
                    eneg = p3.tile([128, F_IN], f32, tag="eneg")
                    nc.scalar.activation(eneg[:], rneg[:], AF.Exp, scale=-1.0)
                    m1 = p3.tile([128, F_IN], f32, tag="m1")
                    nc.vector.scalar_tensor_tensor(
                        out=m1[:], in0=o1[:], scalar=1.0, in1=eneg[:],
                        op0=OP.add, op1=OP.max)
                    h1b = p3.tile([128, F_IN], bf16, tag="h1b")
                    nc.vector.tensor_scalar_add(h1b[:], m1[:], -1.0)
                    p2t = ps2.tile([128, 69], f32)
                    for k in range(4):
                        ptt = pst.tile([128, 128], bf16)
                        nc.tensor.transpose(ptt[:], h1b[:, k * 128:(k + 1) * 128],
                                            ident[:])
                        h1tk = p3.tile([128, 128], bf16, tag="h1tk")
                        nc.scalar.activation(h1tk[:], ptt[:], AF.Copy)
                        nc.tensor.matmul(p2t[:], h1tk[:],
                                         w2t[:, k * 69:(k + 1) * 69],
                                         start=(k == 0), stop=False)
                    nc.tensor.matmul(p2t[:], e0ones[:], w2bt[:],
                                     start=False, stop=True)
                    gt = p3.tile([128, GROW], bf16, tag="gt")
                    nc.scalar.activation(gt[:, 0:64], p2t[:, 0:64], AF.Copy)
                    nc.scalar.activation(gt[:, 64:65], p2t[:, 64:65], AF.Exp)
                    nc.scalar.activation(gt[:, 65:66], p2t[:, 64:65], AF.Exp,
                                         scale=NEG_SLOPE)
                    nc.scalar.activation(gt[:, 66:67], p2t[:, 65:66], AF.Exp)
                    nc.scalar.activation(gt[:, 67:68], p2t[:, 65:66], AF.Exp,
                                         scale=NEG_SLOPE)
                    nc.scalar.activation(gt[:, 68:69], p2t[:, 68:69], AF.Copy)
                    if b < 3:
                        nc.vector.memset(gt[:, 69:GROW], 0.0)
                    nc.scalar.activation(dt2_all[:, b * 2:(b + 1) * 2],
                                         gt[:, 66:68], AF.Copy)
                    nc.scalar.dma_start(Gcat_loc[b * 128:(b + 1) * 128, :], gt[:])
                    for r, (lo, hi) in enumerate(HCH[:-1]):
                        if b == hi - 1:
                            nc.gpsimd.collective_compute(
                                "AllGather", mybir.AluOpType.bypass,
                                replica_groups=groups,
                                ins=[Gcat_loc[lo * BLK:hi * BLK, :]],
                                outs=[Gcat_g[HBASE[r]:HBASE[r] + NCORES
                                             * (hi - lo) * BLK, :]])

                for b in range(NB + 1):
                    if b < NB:
                        stage1(b)
                    if b >= 1:
                        stage2(b - 1)

            # ---- phase 4: AllGather Gcat (tail chunk) ----
            lo, hi = HCH[-1]
            nc.gpsimd.collective_compute(
                "AllGather", mybir.AluOpType.bypass, replica_groups=groups,
                ins=[Gcat_loc[lo * BLK:hi * BLK, :]],
                outs=[Gcat_g[HBASE[-1]:NGLOB, :]])

            # ---- phase 5: L2 edges + log_softmax (2-stage pipeline) ----
            with tc.tile_pool(name="p5", bufs=3) as p5, \
                 tc.tile_pool(name="pg5", bufs=1) as pg5, \
                 tc.tile_pool(name="p5s", bufs=2) as p5s, \
                 tc.tile_pool(name="psuv2", bufs=1, space="PSUM") as psuv2, \
                 tc.tile_pool(name="ps2b", bufs=2, space="PSUM") as ps2b:
                p2of = {}

                def stage5a(b):
                    sbst5 = p5s.tile([128, 2 * CH1 * 128], mybir.dt.int8,
                                     tag="sbst5")
                    nc.sync.dma_start(sbst5[:], SB1[b, :, :])
                    sb5 = sbst5[:, 0:CH1 * 128].bitcast(fp8)
                    st5 = sbst5[:, CH1 * 128:2 * CH1 * 128].bitcast(fp8)
                    g4 = pg5.tile([128, CH1 * GROW], bf16, tag=f"g4{b % 3}")
                    g4r = g4[:].rearrange("p (c w) -> p c w", w=GROW)
                    if b < 3:
                        nc.vector.memset(g4[:], 0.0)
                    for s in (0, 1):
                        nvs = int(nvL1[b, s])
                        if nvs > 0:
                            nc.gpsimd.dma_gather(
                                out_ap=g4r[:, s * CPS:(s + 1) * CPS, :],
                                in_ap=(Gcat_g[0:S0TOT, :] if s == 0 else
                                       Gcat_g[S0TOT:NGLOB, :]),
                                idxs_ap=idx1t[:, (b * 2 + s) * WCI:
                                              (b * 2 + s + 1) * WCI],
                                num_idxs=SIDE, num_idxs_reg=nvregs[nvs],
                                elem_size=GROW, queue_num=(b * 2 + s) % 4)
                    puv2 = psuv2.tile([128, CH1 * 2], f32)
                    for c in range(CH1):
                        nc.tensor.matmul(puv2[:, c * 2:(c + 1) * 2],
                                         st5[:, c * 128:(c + 1) * 128],
                                         dt2_all[:, b * 2:(b + 1) * 2],
                                         start=True, stop=True)
                    uv2b = p5.tile([128, CH1 * 2], bf16, tag="uv2b")
                    nc.scalar.activation(uv2b[:], puv2[:], AF.Copy)
                    uv2v = uv2b[:].rearrange("p (c w) -> p c w", w=2)
                    U2 = p5.tile([128, CH1], f32, tag="U2")
                    V2 = p5.tile([128, CH1], f32, tag="V2")
                    nc.vector.tensor_tensor(
                        out=U2[:].rearrange("p (c o) -> p c o", o=1),
                        in0=g4r[:, :, 64:65], in1=uv2v[:, :, 0:1], op=OP.mult)
                    nc.vector.tensor_tensor(
                        out=V2[:].rearrange("p (c o) -> p c o", o=1),
                        in0=g4r[:, :, 65:66], in1=uv2v[:, :, 1:2], op=OP.mult)
                    ex2 = p5.tile([128, CH1], bf16, tag="ex2")
                    nc.vector.tensor_tensor(out=ex2[:], in0=U2[:], in1=V2[:],
                                            op=OP.max)
                    g4f = p5.tile([128, CH1 * GROW], bf16, tag="g4f")
                    nc.vector.tensor_tensor(
                        out=g4f[:].rearrange("p (c w) -> p c w", w=GROW),
                        in0=g4r,
                        in1=ex2[:].rearrange("p (c o) -> p c o", o=1).to_broadcast(
                            [128, CH1, GROW]),
                        op=OP.mult)
                    g4f3 = g4f[:].rearrange("p (c w) -> p c w", w=GROW)
                    p2b = ps2b.tile([128, 69], f32)
                    for c in range(CH1):
                        nc.tensor.matmul(p2b[:], sb5[:, c * 128:(c + 1) * 128],
                                         g4f3[:, c, 0:69],
                                         start=(c == 0), stop=(c == CH1 - 1))
                    p2of[b] = p2b

                def stage5b(b):
                    p2b = p2of.pop(b)
                    den2 = p5.tile([128, 1], f32, tag="den2")
                    nc.vector.tensor_scalar_add(den2[:], p2b[:, 68:69], EPS)
                    rec2 = p5.tile([128, 1], f32, tag="rec2")
                    nc.vector.reciprocal(rec2[:], den2[:])
                    ozt = p5.tile([128, 128], f32, tag="ozt")
                    Zt = p5.tile([128, 64], f32, tag="Zt")
                    nc.vector.tensor_scalar_mul(Zt[:], p2b[:, 0:64], rec2[:, 0:1])
                    nc.scalar.activation(ozt[:, 64:128], Zt[:], AF.Copy)
                    mx = p5.tile([128, 1], f32, tag="mx")
                    nc.vector.reduce_max(mx[:], Zt[:], axis=mybir.AxisListType.X)
                    nmx = p5.tile([128, 1], f32, tag="nmx")
                    nc.vector.tensor_scalar_mul(nmx[:], mx[:], -1.0)
                    ez = p5.tile([128, 64], f32, tag="ez")
                    nc.scalar.activation(ez[:], Zt[:], AF.Exp, bias=nmx[:, 0:1])
                    sz = p5.tile([128, 1], f32, tag="sz")
                    nc.vector.reduce_sum(sz[:], ez[:], axis=mybir.AxisListType.X)
                    lse = p5.tile([128, 1], f32, tag="lse")
                    nc.scalar.activation(lse[:], sz[:], AF.Ln)
                    nc.vector.tensor_scalar(
                        out=ozt[:, 0:64], in0=Zt[:], scalar1=mx[:, 0:1],
                        scalar2=lse[:, 0:1], op0=OP.subtract, op1=OP.subtract)
                    nc.scalar.dma_start(out_cat[b * 128:(b + 1) * 128, :], ozt[:])

                for b in range(NB + 1):
                    if b < NB:
                        stage5a(b)
                    if b >= 1:
                        stage5b(b - 1)

    mybir.codegen_inst_isa_subclasses(nc)
    return nc


# ---------------- top-level entry ----------------

_CACHE = {}


def kernel(x, edge_index, W1, att_src1, att_dst1, b1, W2, att_src2, att_dst2,
           b2, _trace=False):
    in_maps, meta = _host_prep(x, edge_index, W1, att_src1, att_dst1, b1, W2,
                               att_src2, att_dst2, b2)
    if "prog" not in _CACHE:
        _CACHE["prog"] = _build_program(meta["nvL1"])
    nc = _CACHE["prog"]
    res = run_bass_kernel_spmd(nc, in_maps, list(range(NCORES)), trace=_trace)
    node_of_gid = meta["node_of_gid"]
    out = np.zeros((N, 64), np.float32)
    Z = np.zeros((N, 64), np.float32)
    for c in range(NCORES):
        oc = np.asarray(res.results[c]["out_cat"])
        cols = node_of_gid[c * NLOC:(c + 1) * NLOC]
        valid = cols >= 0
        out[cols[valid]] = oc[valid, 0:64]
        Z[cols[valid]] = oc[valid, 64:128]
    kernel._last_exec_ns = res.exec_time_ns
    kernel._last_res = res
    return (out, Z)



# revision 8
# speedup vs baseline: 1.1506x; 1.1506x over previous
"""GAT 2-layer kernel for 8 trn2 NeuronCores (self-contained).

Sharding: destination-node blocks across 8 cores. Per core: 49 blocks x 128
nodes. Layer GEMMs computed on owned nodes; per-node attention factors
(u=exp(a), v=exp(0.2 a), src factors pair-duplicated) folded into gatherable
row tables (Hcat/Gcat) that are AllGathered; edge phases gather rows by src
via dma_gather (features and attention factors as separate gathers so DVE
ops see packed last dims -> 2x mode) and do segment-softmax-aggregation with
per-chunk selection-matrix matmuls (exp(leaky(x)) == max(exp(x), exp(0.2 x))
makes the logits separable). Edge slots are split by source half (int16
gather indices) with per-(block,side) chunk counts fixed at compile time;
node->block assignment balances per-side edge loads so chunk counts stay
minimal. Layer-2 log_softmax runs as one batched epilogue over all blocks.
"""
import sys, types
sys.path.insert(0, '/opt/trn_rl_repo')

import numpy as np
import ml_dtypes

# ---------------- problem constants (hardcoded) ----------------
N = 50000
F_IN = 512
HID = 64
HEADS = 8
CLS = 64
NEG_SLOPE = 0.2
NCORES = 8
NPC = 6250
NB = 49
BLK = 128
NLOC = NB * BLK           # 6272
NGLOB = NCORES * NLOC     # 50176
HROW = 640                # Hcat row (bf16): 512 feat + 32 src-dup + 16 dst + pad
GROW = 128                # Gcat row (bf16)
NB0 = 25                  # blocks in side 0
R0 = NB0 * BLK            # 3200 rows per core in side 0
R1 = NLOC - R0            # 3072 rows per core in side 1
S0TOT = NCORES * R0       # 25600 rows in side 0
S1TOT = NCORES * R1       # 24576 rows in side 1
HCH = [(0, 13), (13, 25), (25, 37), (37, 49)]  # AG chunks (block ranges)
HBASE = []                # global row base of each AG chunk
_acc = 0
for _lo, _hi in HCH:
    HBASE.append(_acc)
    _acc += NCORES * (_hi - _lo) * BLK
EPS = 1e-16


def _install_ntff_hook():
    if 'antenv.axon_hooks' in sys.modules:
        return
    try:
        sys.path.insert(0, '/root/.axon_site')
        from trn_agent_boot.trn_boot import _ntff_profile_via_ctypes
        hook = _ntff_profile_via_ctypes('/opt/axon/libaxon_pjrt.so')
    except Exception:
        return
    mod = types.ModuleType('antenv.axon_hooks')
    mod._hook = hook
    mod.get_axon_ntff_profile_hook = lambda: mod._hook
    mod.set_axon_ntff_profile_hook = lambda h: setattr(mod, '_hook', h)
    sys.modules['antenv.axon_hooks'] = mod


_install_ntff_hook()

import concourse.bass as bass
import concourse.mybir as mybir
import concourse.tile as tile
from concourse import library_config
from concourse.bass_utils import run_bass_kernel_spmd
from concourse.vector_clock import VectorClock, ScopedClock

bf16 = mybir.dt.bfloat16
f32 = mybir.dt.float32
fp8 = mybir.dt.float8e4

# ------------- tile framework patches (walrus: 1 sync wait / inst) ---------


def _drain_and_barrier_split(self, tick_clock, wait_clock):
    nc = self.nc
    full = tick_clock.global_clock
    procs = [p for p in range(27) if full[p] > 0]
    for p in procs:
        sub = VectorClock([full[q] if q == p else 0 for q in range(27)])
        drain_inst = nc.sync.drain(fusable=False)
        wait_clock.add_sem_waits(drain_inst.ins, ScopedClock({None: sub}))
    if not procs:
        nc.sync.drain(fusable=False)
    nc.all_engine_barrier()
    assert self.sems is not None
    popped = nc._tile_sem_poison_stack.pop()
    assert popped is self._sem_poison
    nc.clear_and_free_semaphores(list(self.sems.allocated().values()))
    nc.all_engine_barrier()


def _split_excess_waits(nc):
    for bb in nc.main_func.blocks:
        insts = bb.instructions
        i = 0
        while i < len(insts):
            ins = insts[i]
            si = ins.sync_info
            if si is None:
                i += 1
                continue
            waits = list(si.on_wait)
            if len(waits) <= 1:
                i += 1
                continue
            keep, surplus = waits[:1], waits[1:]
            ins.sync_info = mybir.SyncInfo(on_wait=keep, on_update=list(si.on_update))
            nops = []
            for w in surplus:
                nop = mybir.InstNoOp(name=nc.get_next_instruction_name())
                nop.engine = ins.engine
                nop.sync_info = mybir.SyncInfo(on_wait=[w], on_update=[])
                nc.register_instruction(nop)
                nops.append(nop)
            for k, nop in enumerate(nops):
                insts.insert(i + k, nop)
            i += 1 + len(nops)


_PATCHED = False


def _install_patches():
    global _PATCHED
    if _PATCHED:
        return
    _orig_exit = tile.TileContext.__exit__

    def _exit_with_split(self, *a):
        r = _orig_exit(self, *a)
        _split_excess_waits(self.nc)
        return r

    tile.TileContext._drain_and_barrier = _drain_and_barrier_split
    tile.TileContext.__exit__ = _exit_with_split
    _PATCHED = True


# ---------------- host-side graph preprocessing ----------------


def _wrap16(flat_idx):
    W = len(flat_idx) // 16
    t = np.asarray(flat_idx, np.int16).reshape(W, 16).T
    return np.tile(t, (8, 1))


def _host_prep(x, edge_index, W1, att_src1, att_dst1, b1, W2, att_src2,
               att_dst2, b2):
    src_o = np.concatenate([np.asarray(edge_index[0]),
                            np.arange(N, dtype=np.int64)]).astype(np.int64)
    dst_o = np.concatenate([np.asarray(edge_index[1]),
                            np.arange(N, dtype=np.int64)]).astype(np.int64)

    core_of = dst_o // NPC
    deg = np.bincount(dst_o, minlength=N)

    # ---- pass 1: degree-balanced LPT to fix each node's half ----
    half_of = np.full(N, -1, np.int64)   # 0 if in blocks [0,25), 1 else
    for c in range(NCORES):
        nodes = np.arange(c * NPC, (c + 1) * NPC)
        d = deg[nodes]
        order = np.argsort(-d, kind='stable')
        blk_cnt = np.zeros(NB, np.int64)
        blk_load = np.zeros(NB, np.int64)
        for i in order:
            b = int(np.argmin(blk_load + (blk_cnt >= BLK) * (1 << 40)))
            blk_cnt[b] += 1
            blk_load[b] += d[i]
            half_of[nodes[i]] = 0 if b < NB0 else 1

    # edge sides now fixed by src half
    e_side = half_of[src_o]

    # per-node in-edge counts split by side (self-loops included in dst_o)
    d0 = np.bincount(dst_o[e_side == 0], minlength=N)
    d1 = np.bincount(dst_o[e_side == 1], minlength=N)

    # ---- pass 2: within each half, rebalance blocks on (d0, d1) ----
    gid = np.full(N, -1, np.int64)
    node_of_gid = np.full(NGLOB, -1, np.int64)
    for c in range(NCORES):
        nodes = np.arange(c * NPC, (c + 1) * NPC)
        for h, (blo, bhi) in ((0, (0, NB0)), (1, (NB0, NB))):
            hn = nodes[half_of[nodes] == h]
            nb_h = bhi - blo
            l0 = np.zeros(nb_h, np.int64)
            l1 = np.zeros(nb_h, np.int64)
            cnt = np.zeros(nb_h, np.int64)
            order = np.argsort(-(d0[hn] + d1[hn]), kind='stable')
            slot_ctr = np.zeros(nb_h, np.int64)
            for i in order:
                n = hn[i]
                pen = (cnt >= BLK) * (1 << 40)
                cost = np.maximum(l0 + d0[n], l1 + d1[n]) + pen
                b = int(np.argmin(cost * (1 << 20) + (l0 + l1)))
                cnt[b] += 1
                l0[b] += d0[n]
                l1[b] += d1[n]
                g = c * NLOC + (blo + b) * BLK + slot_ctr[b]
                slot_ctr[b] += 1
                gid[n] = g
                node_of_gid[g] = n

    src_g = gid[src_o]
    dst_g = gid[dst_o]
    dst_block = (dst_g % NLOC) // BLK
    dst_slot = dst_g % BLK
    src_core = src_g // NLOC
    src_row = src_g % NLOC
    src_blk = src_row // BLK
    side = (src_row >= R0).astype(np.int64)
    src_gidx = np.zeros_like(src_g)
    for _r, (_lo, _hi) in enumerate(HCH):
        _m = (src_blk >= _lo) & (src_blk < _hi)
        src_gidx[_m] = (HBASE[_r] + src_core[_m] * (_hi - _lo) * BLK
                        + (src_row[_m] - _lo * BLK))
    src_sidx = np.where(side == 0, src_gidx, src_gidx - S0TOT)

    # ---- per-(core, block, side) edge lists ----
    SIDE_MAX = 768
    idxL1 = np.full((NCORES, NB, 2, SIDE_MAX), 0, np.int32)
    dslL1 = np.full((NCORES, NB, 2, SIDE_MAX), -1, np.int32)
    cntL1 = np.zeros((NCORES, NB, 2), np.int64)
    for c in range(NCORES):
        em = np.nonzero(core_of == c)[0]
        eb = dst_block[em]
        for b in range(NB):
            eidx = em[eb == b]
            dslots = dst_slot[eidx]
            sides = side[eidx]
            for s in (0, 1):
                ms = sides == s
                k = int(ms.sum())
                if k > SIDE_MAX:
                    raise RuntimeError(f"L1 overflow c{c} b{b} s{s}: {k}")
                idxL1[c, b, s, :k] = src_sidx[eidx][ms]
                dslL1[c, b, s, :k] = dslots[ms]
                cntL1[c, b, s] = k

    nvmax = cntL1.max(axis=0)                       # [NB, 2]
    nv64 = np.minimum(((nvmax + 63) // 64) * 64, SIDE_MAX).astype(np.int64)
    nv64 = np.maximum(nv64, 64)
    cps = ((nv64 + 127) // 128).astype(np.int64)    # chunks per (b, s)
    chb = cps.sum(axis=1)                           # chunks per block

    # ---- selection tables: per block [128, 2*CH*128] int8 (sb | stb) ----
    sb_off = np.zeros(NB + 1, np.int64)             # col offset into flat table
    for b in range(NB):
        sb_off[b + 1] = sb_off[b] + 2 * chb[b] * 128
    SBFLAT = np.zeros((NCORES, 128, int(sb_off[NB])), np.float32)
    for c in range(NCORES):
        for b in range(NB):
            ch_n = int(chb[b])
            sl = np.zeros((128, ch_n * 128), np.float32)
            st = np.zeros((128, ch_n * 128), np.float32)
            for s in (0, 1):
                base_ch = 0 if s == 0 else int(cps[b, 0])
                k = int(cntL1[c, b, s])
                for j in range(k):
                    ch = base_ch + j // 128
                    e_i = j % 128
                    n_i = int(dslL1[c, b, s, j])
                    sl[e_i, ch * 128 + n_i] = 1
                    st[n_i, ch * 128 + e_i] = 1
            SBFLAT[c, :, int(sb_off[b]):int(sb_off[b]) + ch_n * 128] = sl
            SBFLAT[c, :, int(sb_off[b]) + ch_n * 128:int(sb_off[b + 1])] = st
    SBFLAT = SBFLAT.astype(ml_dtypes.float8_e4m3).view(np.int8)

    # ---- int16 gather index table, wrapped 16, per-(b,s) offsets ----
    ix_off = np.zeros((NB, 2), np.int64)
    _o = 0
    for b in range(NB):
        for s in (0, 1):
            ix_off[b, s] = _o
            _o += int(nv64[b, s]) // 16
    IXW = int(_o)
    idx_dev = np.zeros((NCORES, 128, IXW), np.int16)
    for c in range(NCORES):
        for b in range(NB):
            for s in (0, 1):
                nv = int(nv64[b, s])
                col = int(ix_off[b, s])
                idx_dev[c, :, col:col + nv // 16] = _wrap16(idxL1[c, b, s, :nv])

    # ---- weights ----
    W1 = np.asarray(W1, np.float32)
    att_src1 = np.asarray(att_src1, np.float32)
    att_dst1 = np.asarray(att_dst1, np.float32)
    b1 = np.asarray(b1, np.float32)
    W2 = np.asarray(W2, np.float32)
    att_src2 = np.asarray(att_src2, np.float32)
    att_dst2 = np.asarray(att_dst2, np.float32)
    b2 = np.asarray(b2, np.float32)

    Msrc = np.zeros((F_IN, HEADS), np.float32)
    Mdst = np.zeros((F_IN, HEADS), np.float32)
    for h in range(HEADS):
        Msrc[h * HID:(h + 1) * HID, h] = att_src1[h]
        Mdst[h * HID:(h + 1) * HID, h] = att_dst1[h]
    W1aug = np.zeros((F_IN + 128, F_IN + 16), np.float32)
    W1aug[:F_IN, 0:F_IN] = W1
    W1aug[:F_IN, F_IN:F_IN + 8] = W1 @ Msrc
    W1aug[:F_IN, F_IN + 8:F_IN + 16] = W1 @ Mdst
    W1aug[F_IN, 0:F_IN] = b1
    W2aug = np.zeros((F_IN, 69), np.float32)
    W2aug[:, 0:CLS] = W2
    W2aug[:, 64] = W2 @ att_src2[0]
    W2aug[:, 65] = W2 @ att_dst2[0]
    W2bias = np.zeros((128, 69), np.float32)
    W2bias[0, 0:CLS] = b2
    W2bias[0, 68] = 1.0

    x = np.asarray(x, np.float32)
    in_maps = []
    W1aug_bf = np.ascontiguousarray(W1aug.astype(ml_dtypes.bfloat16))
    W2aug_bf = np.ascontiguousarray(W2aug.astype(ml_dtypes.bfloat16))
    W2bias_bf = np.ascontiguousarray(W2bias.astype(ml_dtypes.bfloat16))
    for c in range(NCORES):
        cols = node_of_gid[c * NLOC:(c + 1) * NLOC]
        validc = cols >= 0
        xc = np.zeros((NLOC, F_IN), np.float32)
        xc[validc] = x[cols[validc]]
        xt = np.zeros((F_IN + 128, NLOC), np.float32)
        xt[:F_IN] = xc.T
        xt[F_IN] = 1.0
        in_maps.append({
            "xT": np.ascontiguousarray(xt.astype(ml_dtypes.bfloat16)),
            "W1aug": W1aug_bf,
            "W2aug": W2aug_bf,
            "W2bias": W2bias_bf,
            "idxT": np.ascontiguousarray(idx_dev[c]),
            "SBF": np.ascontiguousarray(SBFLAT[c]),
        })
    meta = {"node_of_gid": node_of_gid, "nv64": nv64, "cps": cps,
            "sb_off": sb_off, "ix_off": ix_off, "IXW": IXW}
    return in_maps, meta


# ---------------- device program ----------------


def _build_program(meta):
    _install_patches()
    nv64 = meta["nv64"]
    cps = meta["cps"]
    sb_off = meta["sb_off"]
    ix_off = meta["ix_off"]
    IXW = meta["IXW"]
    chb = cps.sum(axis=1)
    CHMAX = int(chb.max())

    nc = bass.Bass(num_swdge_queues=4)
    AF = mybir.ActivationFunctionType
    OP = mybir.AluOpType
    KW = (F_IN + 128) // 128          # 5 k-chunks for GEMM1
    WROW = F_IN + 16                  # 528 W1aug cols

    xT = nc.dram_tensor("xT", [F_IN + 128, NLOC], bf16, kind="ExternalInput")
    W1a = nc.dram_tensor("W1aug", [F_IN + 128, WROW], bf16, kind="ExternalInput")
    W2a = nc.dram_tensor("W2aug", [F_IN, 69], bf16, kind="ExternalInput")
    W2b = nc.dram_tensor("W2bias", [128, 69], bf16, kind="ExternalInput")
    idxT = nc.dram_tensor("idxT", [128, IXW], mybir.dt.int16,
                          kind="ExternalInput")
    SBF = nc.dram_tensor("SBF", [128, int(sb_off[NB])], mybir.dt.int8,
                         kind="ExternalInput")

    out_cat = nc.dram_tensor("out_cat", [NLOC, 128], f32, kind="ExternalOutput")

    Hcat_loc = nc.dram_tensor("Hcat_loc", [NLOC, HROW], bf16)
    Hcat_g = nc.dram_tensor("Hcat_g", [NGLOB, HROW], bf16, addr_space="Shared")
    Gcat_loc = nc.dram_tensor("Gcat_loc", [NLOC, GROW], bf16)
    Gcat_g = nc.dram_tensor("Gcat_g", [NGLOB, GROW], bf16, addr_space="Shared")

    groups = [list(range(NCORES))]

    with tile.TileContext(nc) as tc:
        with tc.tile_critical():
            nc.gpsimd.load_library(library_config.mlp)
        nvset = sorted({int(v) for v in nv64.flatten()})
        nvregs = {v: nc.gpsimd.to_reg(v) for v in nvset if v > 0}

        with tc.tile_pool(name="const", bufs=1) as constp:
            w1t = constp.tile([128, KW * WROW], bf16)
            for k in range(KW):
                nc.sync.dma_start(w1t[:, k * WROW:(k + 1) * WROW],
                                  W1a[k * 128:(k + 1) * 128, :])
            w2t = constp.tile([128, 4 * 69], bf16)
            for k in range(4):
                nc.sync.dma_start(w2t[:, k * 69:(k + 1) * 69],
                                  W2a[k * 128:(k + 1) * 128, :])
            w2bt = constp.tile([128, 69], bf16)
            nc.sync.dma_start(w2bt[:], W2b[:, :])
            e0ones = constp.tile([128, 128], bf16)
            nc.vector.memset(e0ones[:], 0.0)
            nc.vector.memset(e0ones[0:1, :], 1.0)
            ident = constp.tile([128, 128], bf16)
            from concourse.masks import make_identity
            make_identity(nc, ident[:])
            dt32 = constp.tile([128, NB * 32], bf16)   # dup'd dst factors L1
            dt2_all = constp.tile([128, NB * 2], bf16)  # dst factors L2
            idx1t = constp.tile([128, IXW], mybir.dt.int16)
            nc.sync.dma_start(idx1t[:], idxT[:, :])

            # ---- phase 1: GEMM1 + Hcat rows ----
            with tc.tile_pool(name="p1", bufs=3) as p1, \
                 tc.tile_pool(name="ps1a", bufs=2, space="PSUM") as ps1a, \
                 tc.tile_pool(name="ps1b", bufs=2, space="PSUM") as ps1b:
                for b in range(NB):
                    pA = ps1a.tile([128, F_IN], f32)
                    pB = ps1b.tile([128, 16], f32)
                    xt = p1.tile([128, KW * 128], bf16, tag="xt")
                    nc.sync.dma_start(
                        xt[:].rearrange("p (k j) -> p k j", k=KW),
                        xT[0:KW * 128, b * 128:(b + 1) * 128].rearrange(
                            "(k p) j -> p k j", k=KW))
                    for k in range(KW):
                        nc.tensor.matmul(pA[:], xt[:, k * 128:(k + 1) * 128],
                                         w1t[:, k * WROW:k * WROW + F_IN],
                                         start=(k == 0), stop=(k == KW - 1))
                        nc.tensor.matmul(pB[:], xt[:, k * 128:(k + 1) * 128],
                                         w1t[:, k * WROW + F_IN:(k + 1) * WROW],
                                         start=(k == 0), stop=(k == KW - 1))
                    hc = p1.tile([128, HROW], bf16, tag="hc")
                    nc.scalar.activation(hc[:, 0:F_IN], pA[:], AF.Copy)
                    # src factors, pair-duplicated: u = e^{a_src}, v = e^{.2 a_src}
                    asrc2 = pB[:, 0:8].rearrange("p (k o) -> p k o", o=1) \
                        .to_broadcast([128, 8, 2])
                    nc.scalar.activation(
                        hc[:, 512:528].rearrange("p (k t) -> p k t", t=2),
                        asrc2, AF.Exp)
                    nc.scalar.activation(
                        hc[:, 528:544].rearrange("p (k t) -> p k t", t=2),
                        asrc2, AF.Exp, scale=NEG_SLOPE)
                    # dst factors (not duplicated in the row)
                    nc.scalar.activation(hc[:, 544:552], pB[:, 8:16], AF.Exp)
                    nc.scalar.activation(hc[:, 552:560], pB[:, 8:16], AF.Exp,
                                         scale=NEG_SLOPE)
                    if b < 3:
                        nc.vector.memset(hc[:, 560:HROW], 0.0)
                    # dup'd dst factor table for the puv matmuls
                    nc.scalar.activation(
                        dt32[:, b * 32:(b + 1) * 32].rearrange(
                            "p (k t) -> p k t", t=2),
                        hc[:, 544:560].rearrange("p (k o) -> p k o", o=1)
                        .to_broadcast([128, 16, 2]),
                        AF.Copy)
                    nc.scalar.dma_start(Hcat_loc[b * 128:(b + 1) * 128, :], hc[:])
                    for r, (lo, hi) in enumerate(HCH[:-1]):
                        if b == hi - 1:
                            nc.gpsimd.collective_compute(
                                "AllGather", mybir.AluOpType.bypass,
                                replica_groups=groups,
                                ins=[Hcat_loc[lo * BLK:hi * BLK, :]],
                                outs=[Hcat_g[HBASE[r]:HBASE[r] + NCORES
                                             * (hi - lo) * BLK, :]])

            # ---- phase 2: AllGather Hcat (tail chunk) ----
            lo, hi = HCH[-1]
            nc.gpsimd.collective_compute(
                "AllGather", mybir.AluOpType.bypass, replica_groups=groups,
                ins=[Hcat_loc[lo * BLK:hi * BLK, :]],
                outs=[Hcat_g[HBASE[-1]:NGLOB, :]])

            # ---- phase 3: L1 edges + block tails + GEMM2 + Gcat ----
            with tc.tile_pool(name="p3", bufs=3) as p3, \
                 tc.tile_pool(name="pg3", bufs=1) as pg3, \
                 tc.tile_pool(name="p3s", bufs=2) as p3s, \
                 tc.tile_pool(name="psA", bufs=2, space="PSUM") as psA, \
                 tc.tile_pool(name="psB", bufs=2, space="PSUM") as psB, \
                 tc.tile_pool(name="psuv", bufs=2, space="PSUM") as psuv, \
                 tc.tile_pool(name="pst", bufs=1, space="PSUM") as pst, \
                 tc.tile_pool(name="ps2", bufs=1, space="PSUM") as ps2:
                pAB = {}
                gtiles = {}

                def gather3(b):
                    gc = pg3.tile([128, CHMAX * F_IN], bf16, tag=f"gc{b % 3}")
                    ga = pg3.tile([128, CHMAX * 128], bf16, tag=f"ga{b % 3}")
                    if b < 3:
                        nc.vector.memset(gc[:], 0.0)
                        nc.vector.memset(ga[:], 0.0)
                    for s in (0, 1):
                        nv = int(nv64[b, s])
                        cp = int(cps[b, s])
                        c0 = 0 if s == 0 else int(cps[b, 0])
                        icol = int(ix_off[b, s])
                        iap = idx1t[:, icol:icol + nv // 16]
                        hin = Hcat_g[0:S0TOT, :] if s == 0 else \
                            Hcat_g[S0TOT:NGLOB, :]
                        nc.gpsimd.dma_gather(
                            out_ap=gc[:, c0 * F_IN:(c0 + cp) * F_IN].rearrange(
                                "p (c w) -> p c w", w=F_IN),
                            in_ap=hin[:, 0:F_IN],
                            idxs_ap=iap, num_idxs=nv,
                            num_idxs_reg=nvregs[nv], elem_size=F_IN,
                            elem_step=HROW, queue_num=(b * 4 + s) % 4)
                        nc.gpsimd.dma_gather(
                            out_ap=ga[:, c0 * 128:(c0 + cp) * 128].rearrange(
                                "p (c w) -> p c w", w=128),
                            in_ap=hin[:, 512:640],
                            idxs_ap=iap, num_idxs=nv,
                            num_idxs_reg=nvregs[nv], elem_size=128,
                            elem_step=HROW, queue_num=(b * 4 + 2 + s) % 4)
                    gtiles[b] = (gc, ga)

                def stage1(b):
                    CH = int(chb[b])
                    gc, ga = gtiles.pop(b)
                    sbst = p3s.tile([128, 2 * CHMAX * 128], mybir.dt.int8,
                                    tag="sbst")
                    nc.sync.dma_start(sbst[:, 0:2 * CH * 128],
                                      SBF[:, int(sb_off[b]):int(sb_off[b + 1])])
                    sb = sbst[:, 0:CH * 128].bitcast(fp8)
                    stb = sbst[:, CH * 128:2 * CH * 128].bitcast(fp8)
                    # per-edge dst factors (dup'd), via selection matmuls
                    puv = psuv.tile([128, CHMAX * 32], f32)
                    for c in range(CH):
                        nc.tensor.matmul(puv[:, c * 32:(c + 1) * 32],
                                         stb[:, c * 128:(c + 1) * 128],
                                         dt32[:, b * 32:(b + 1) * 32],
                                         start=True, stop=True)
                    uvb = p3.tile([128, CHMAX * 32], bf16, tag="uvb")
                    nc.scalar.activation(uvb[:, 0:CH * 32], puv[:, 0:CH * 32],
                                         AF.Copy)
                    # tuv = src factors * dst factors (all pair-dup'd) -> 2x
                    tuv = p3.tile([128, CHMAX * 32], bf16, tag="tuv")
                    nc.vector.tensor_tensor(
                        out=tuv[:, 0:CH * 32].rearrange("p (c w) -> p c w", w=32),
                        in0=ga[:, 0:CH * 128].rearrange(
                            "p (c w) -> p c w", w=128)[:, :, 0:32],
                        in1=uvb[:, 0:CH * 32].rearrange("p (c w) -> p c w", w=32),
                        op=OP.mult)
                    # exb = max(u-part, v-part), stays pair-dup'd
                    exb = p3.tile([128, CHMAX * 16], bf16, tag="exb")
                    nc.vector.tensor_tensor(
                        out=exb[:, 0:CH * 16].rearrange("p (c w) -> p c w", w=16),
                        in0=tuv[:, 0:CH * 32].rearrange(
                            "p (c w) -> p c w", w=32)[:, :, 0:16],
                        in1=tuv[:, 0:CH * 32].rearrange(
                            "p (c w) -> p c w", w=32)[:, :, 16:32],
                        op=OP.max)
                    # hs = gathered features * exb (head-broadcast, packed pairs)
                    hs = p3.tile([128, CHMAX * F_IN], bf16, tag="hs")
                    nc.vector.tensor_tensor(
                        out=hs[:, 0:CH * F_IN].rearrange(
                            "p (ch w t) -> p ch w t", w=32, t=2),
                        in0=gc[:, 0:CH * F_IN].rearrange(
                            "p (ch w t) -> p ch w t", w=32, t=2),
                        in1=exb[:, 0:CH * 16].rearrange(
                            "p (ch o t) -> p ch o t", o=1, t=2)
                        .to_broadcast([128, CH * 8, 32, 2]),
                        op=OP.mult)
                    pA = psA.tile([128, F_IN], f32)
                    pB = psB.tile([128, 16], f32)
                    for c in range(CH):
                        nc.tensor.matmul(pA[:], sb[:, c * 128:(c + 1) * 128],
                                         hs[:, c * F_IN:(c + 1) * F_IN],
                                         start=(c == 0), stop=(c == CH - 1))
                        nc.tensor.matmul(pB[:], sb[:, c * 128:(c + 1) * 128],
                                         exb[:, c * 16:(c + 1) * 16],
                                         start=(c == 0), stop=(c == CH - 1))
                    pAB[b] = (pA, pB)

                def stage2(b):
                    pA, pB = pAB.pop(b)
                    sden = p3.tile([128, 16], f32, tag="sden")
                    nc.scalar.activation(sden[:], pB[:], AF.Copy, bias=EPS)
                    rec = p3.tile([128, 16], f32, tag="rec")
                    nc.vector.reciprocal(rec[:], sden[:])
                    o1 = p3.tile([128, F_IN], f32, tag="o1")
                    nc.vector.tensor_tensor(
                        out=o1[:].rearrange("p (h w t) -> p h w t", w=32, t=2),
                        in0=pA[:].rearrange("p (h w t) -> p h w t", w=32, t=2),
                        in1=rec[:].rearrange("p (h o t) -> p h o t", o=1, t=2)
                        .to_broadcast([128, 8, 32, 2]),
                        op=OP.mult)
                    rneg = p3.tile([128, F_IN], f32, tag="rneg")
                    nc.scalar.activation(rneg[:], o1[:], AF.Relu, scale=-1.0)
                    eneg = p3.tile([128, F_IN], f32, tag="eneg")
                    nc.scalar.activation(eneg[:], rneg[:], AF.Exp, scale=-1.0)
                    m1 = p3.tile([128, F_IN], f32, tag="m1")
                    nc.vector.scalar_tensor_tensor(
                        out=m1[:], in0=o1[:], scalar=1.0, in1=eneg[:],
                        op0=OP.add, op1=OP.max)
                    h1b = p3.tile([128, F_IN], bf16, tag="h1b")
                    nc.scalar.activation(h1b[:], m1[:], AF.Copy, bias=-1.0)
                    p2t = ps2.tile([128, 69], f32)
                    for k in range(4):
                        ptt = pst.tile([128, 128], bf16)
                        nc.tensor.transpose(ptt[:], h1b[:, k * 128:(k + 1) * 128],
                                            ident[:])
                        h1tk = p3.tile([128, 128], bf16, tag="h1tk")
                        nc.scalar.activation(h1tk[:], ptt[:], AF.Copy)
                        nc.tensor.matmul(p2t[:], h1tk[:],
                                         w2t[:, k * 69:(k + 1) * 69],
                                         start=(k == 0), stop=False)
                    nc.tensor.matmul(p2t[:], e0ones[:], w2bt[:],
                                     start=False, stop=True)
                    gt = p3.tile([128, GROW], bf16, tag="gt")
                    nc.scalar.activation(gt[:, 0:69], p2t[:, 0:69], AF.Copy)
                    # overwrite 64:68 with the four exp factors:
                    # gt[64,66] = exp(p2t[64,65]); gt[65,67] = exp(.2 p2t[64,65])
                    g4x = gt[:, 64:68].rearrange("p (k t) -> p k t", t=2)
                    p2x = p2t[:, 64:66].rearrange("p (k o) -> p k o", o=1)
                    nc.scalar.activation(g4x[:, :, 0:1], p2x, AF.Exp)
                    nc.scalar.activation(g4x[:, :, 1:2], p2x, AF.Exp,
                                         scale=NEG_SLOPE)
                    if b < 3:
                        nc.vector.memset(gt[:, 69:GROW], 0.0)
                    nc.scalar.activation(dt2_all[:, b * 2:(b + 1) * 2],
                                         gt[:, 66:68], AF.Copy)
                    nc.scalar.dma_start(Gcat_loc[b * 128:(b + 1) * 128, :], gt[:])
                    for r, (lo, hi) in enumerate(HCH[:-1]):
                        if b == hi - 1:
                            nc.gpsimd.collective_compute(
                                "AllGather", mybir.AluOpType.bypass,
                                replica_groups=groups,
                                ins=[Gcat_loc[lo * BLK:hi * BLK, :]],
                                outs=[Gcat_g[HBASE[r]:HBASE[r] + NCORES
                                             * (hi - lo) * BLK, :]])

                gather3(0)
                gather3(1)
                for b in range(NB + 1):
                    if b < NB:
                        stage1(b)
                        if b + 2 < NB:
                            gather3(b + 2)
                    if b >= 1:
                        stage2(b - 1)

            # ---- phase 4: AllGather Gcat (tail chunk) ----
            lo, hi = HCH[-1]
            nc.gpsimd.collective_compute(
                "AllGather", mybir.AluOpType.bypass, replica_groups=groups,
                ins=[Gcat_loc[lo * BLK:hi * BLK, :]],
                outs=[Gcat_g[HBASE[-1]:NGLOB, :]])

            # ---- phase 5: L2 edges, batched log_softmax epilogue ----
            with tc.tile_pool(name="p5", bufs=3) as p5, \
                 tc.tile_pool(name="pg5", bufs=1) as pg5, \
                 tc.tile_pool(name="p5s", bufs=2) as p5s, \
                 tc.tile_pool(name="pcat", bufs=1) as pcat, \
                 tc.tile_pool(name="psuv2", bufs=2, space="PSUM") as psuv2, \
                 tc.tile_pool(name="ps2b", bufs=2, space="PSUM") as ps2b:
                zcat = pcat.tile([128, NB * 64], bf16)
                dcat = pcat.tile([128, NB], f32)
                g5tiles = {}
                p2of = {}

                def gather5(b):
                    g4 = pg5.tile([128, CHMAX * GROW], bf16, tag=f"g4{b % 3}")
                    if b < 3:
                        nc.vector.memset(g4[:], 0.0)
                    for s in (0, 1):
                        nv = int(nv64[b, s])
                        cp = int(cps[b, s])
                        c0 = 0 if s == 0 else int(cps[b, 0])
                        icol = int(ix_off[b, s])
                        nc.gpsimd.dma_gather(
                            out_ap=g4[:, c0 * GROW:(c0 + cp) * GROW].rearrange(
                                "p (c w) -> p c w", w=GROW),
                            in_ap=(Gcat_g[0:S0TOT, :] if s == 0 else
                                   Gcat_g[S0TOT:NGLOB, :]),
                            idxs_ap=idx1t[:, icol:icol + nv // 16],
                            num_idxs=nv, num_idxs_reg=nvregs[nv],
                            elem_size=GROW, queue_num=(b * 2 + s) % 4)
                    g5tiles[b] = g4

                def stage5a(b):
                    CH = int(chb[b])
                    g4 = g5tiles.pop(b)
                    g4r = g4[:, 0:CH * GROW].rearrange("p (c w) -> p c w", w=GROW)
                    sbst5 = p5s.tile([128, 2 * CHMAX * 128], mybir.dt.int8,
                                     tag="sbst5")
                    nc.sync.dma_start(sbst5[:, 0:2 * CH * 128],
                                      SBF[:, int(sb_off[b]):int(sb_off[b + 1])])
                    sb5 = sbst5[:, 0:CH * 128].bitcast(fp8)
                    st5 = sbst5[:, CH * 128:2 * CH * 128].bitcast(fp8)
                    puv2 = psuv2.tile([128, CHMAX * 2], f32)
                    for c in range(CH):
                        nc.tensor.matmul(puv2[:, c * 2:(c + 1) * 2],
                                         st5[:, c * 128:(c + 1) * 128],
                                         dt2_all[:, b * 2:(b + 1) * 2],
                                         start=True, stop=True)
                    uv2b = p5.tile([128, CHMAX * 2], bf16, tag="uv2b")
                    nc.scalar.activation(uv2b[:, 0:CH * 2], puv2[:, 0:CH * 2],
                                         AF.Copy)
                    # tuv2 = src factors (cols 64:66) * dst factors -> 2x
                    tuv2 = p5.tile([128, CHMAX * 2], bf16, tag="tuv2")
                    nc.vector.tensor_tensor(
                        out=tuv2[:, 0:CH * 2].rearrange("p (c t) -> p c t", t=2),
                        in0=g4r[:, :, 64:66],
                        in1=uv2b[:, 0:CH * 2].rearrange("p (c t) -> p c t", t=2),
                        op=OP.mult)
                    # ex2d = max(U, V), pair-duplicated for the g4f multiply
                    ex2d = p5.tile([128, CHMAX * 2], bf16, tag="ex2d")
                    tv = tuv2[:, 0:CH * 2].rearrange("p (c t) -> p c t", t=2)
                    nc.vector.tensor_tensor(
                        out=ex2d[:, 0:CH * 2].rearrange("p (c t) -> p c t", t=2),
                        in0=tv[:, :, 0:1].to_broadcast([128, CH, 2]),
                        in1=tv[:, :, 1:2].to_broadcast([128, CH, 2]),
                        op=OP.max)
                    g4f = p5.tile([128, CHMAX * 72], bf16, tag="g4f")
                    nc.vector.tensor_tensor(
                        out=g4f[:, 0:CH * 72].rearrange(
                            "p (c w t) -> p c w t", w=36, t=2),
                        in0=g4[:, 0:CH * GROW].rearrange(
                            "p (c w) -> p c w", w=GROW)[:, :, 0:72].rearrange(
                            "p c (w t) -> p c w t", t=2),
                        in1=ex2d[:, 0:CH * 2].rearrange(
                            "p (c o t) -> p c o t", o=1, t=2)
                        .to_broadcast([128, CH, 36, 2]),
                        op=OP.mult)
                    p2b = ps2b.tile([128, 69], f32)
                    for c in range(CH):
                        nc.tensor.matmul(p2b[:], sb5[:, c * 128:(c + 1) * 128],
                                         g4f[:, c * 72:c * 72 + 69],
                                         start=(c == 0), stop=(c == CH - 1))
                    p2of[b] = p2b

                def stage5b(b):
                    p2b = p2of.pop(b)
                    nc.scalar.activation(zcat[:, b * 64:(b + 1) * 64],
                                         p2b[:, 0:64], AF.Copy)
                    nc.scalar.activation(dcat[:, b:b + 1], p2b[:, 68:69],
                                         AF.Copy, bias=EPS)

                gather5(0)
                gather5(1)
                for b in range(NB + 1):
                    if b < NB:
                        stage5a(b)
                        if b + 2 < NB:
                            gather5(b + 2)
                    if b >= 1:
                        stage5b(b - 1)

                # batched epilogue: Z = zcat/den, log_softmax without max-sub
                # (|Z| is O(1) for this regime; exp is safe in f32)
                rec = pcat.tile([128, NB], f32)
                nc.vector.reciprocal(rec[:], dcat[:])
                recd = pcat.tile([128, NB * 2], bf16)
                nc.scalar.activation(
                    recd[:].rearrange("p (b t) -> p b t", t=2),
                    rec[:].rearrange("p (b o) -> p b o", o=1)
                    .to_broadcast([128, NB, 2]),
                    AF.Copy)
                Zt = pcat.tile([128, NB * 64], bf16)
                nc.vector.tensor_tensor(
                    out=Zt[:].rearrange("p (b w t) -> p b w t", w=32, t=2),
                    in0=zcat[:].rearrange("p (b w t) -> p b w t", w=32, t=2),
                    in1=recd[:].rearrange("p (b o t) -> p b o t", o=1, t=2)
                    .to_broadcast([128, NB, 32, 2]),
                    op=OP.mult)
                ocat = pcat.tile([128, NB * 128], f32)
                ocv = ocat[:].rearrange("p (b w) -> p b w", w=128)
                nc.scalar.activation(ocv[:, :, 64:128],
                                     Zt[:].rearrange("p (b w) -> p b w", w=64),
                                     AF.Copy)
                ez = pcat.tile([128, NB * 64], f32)
                nc.scalar.activation(ez[:], Zt[:], AF.Exp)
                sz = pcat.tile([128, NB], f32)
                nc.vector.reduce_sum(
                    sz[:],
                    ez[:].rearrange("p (b w) -> p b w", w=64),
                    axis=mybir.AxisListType.X)
                lse = pcat.tile([128, NB], f32)
                nc.scalar.activation(lse[:], sz[:], AF.Ln)
                nc.vector.tensor_tensor(
                    out=ocv[:, :, 0:64],
                    in0=Zt[:].rearrange("p (b w) -> p b w", w=64),
                    in1=lse[:].rearrange("p (b o) -> p b o", o=1)
                    .to_broadcast([128, NB, 64]),
                    op=OP.subtract)
                HB = NB // 2
                nc.sync.dma_start(
                    out_cat[0:HB * 128, :].rearrange("(b p) w -> p b w", p=128),
                    ocv[:, 0:HB, :])
                nc.scalar.dma_start(
                    out_cat[HB * 128:NLOC, :].rearrange("(b p) w -> p b w", p=128),
                    ocv[:, HB:NB, :])

    mybir.codegen_inst_isa_subclasses(nc)
    return nc


# ---------------- top-level entry ----------------

_CACHE = {}


def kernel(x, edge_index, W1, att_src1, att_dst1, b1, W2, att_src2, att_dst2,
           b2, _trace=False):
    in_maps, meta = _host_prep(x, edge_index, W1, att_src1, att_dst1, b1, W2,
                               att_src2, att_dst2, b2)
    if "prog" not in _CACHE:
        _CACHE["prog"] = _build_program(meta)
    nc = _CACHE["prog"]
    res = run_bass_kernel_spmd(nc, in_maps, list(range(NCORES)), trace=_trace)
    node_of_gid = meta["node_of_gid"]
    out = np.zeros((N, 64), np.float32)
    Z = np.zeros((N, 64), np.float32)
    for c in range(NCORES):
        oc = np.asarray(res.results[c]["out_cat"])
        cols = node_of_gid[c * NLOC:(c + 1) * NLOC]
        valid = cols >= 0
        out[cols[valid]] = oc[valid, 0:64]
        Z[cols[valid]] = oc[valid, 64:128]
    kernel._last_exec_ns = res.exec_time_ns
    kernel._last_res = res
    return (out, Z)
